# revision 25
# baseline (speedup 1.0000x reference)
"""Distributed L1-attention dictionary lookup (retrieval_knn) on 8 trn2 cores.

out = sigmoid(softmax(-sum_f |keys - q|) @ values)    (capacity 262144, F 512)

Sharding: capacity split row-wise into 8 shards of 32768 rows (keys+values);
query replicated. Each core runs one fused streaming kernel:

  t_c  = sum_f |k_cf - q_f|     DVE subtract + ScalarE Abs(accum) / DVE |.|-reduce
  m    = min(t, chunk 0) - 20   softmax reference point (any reference is valid
                                for the host combine; margin guards exp overflow)
  a_c  = exp(m - t_c)           ScalarE, per chunk as scores land
  num  = sum_c a_c * v_c        PE, 256 accumulating [128,1]x[128,512] matmuls
  s    = sum_c a_c

Keys and values stream together in 512-row chunks (one 1 MiB DMA per tensor
per chunk, per-partition-contiguous layout, same permutation for both so the
attention/value pairing stays consistent). Everything overlaps: steady state
is HBM-bandwidth-bound (~358 GB/s/core), measured ~359 us/iteration =
~1.0x the 128 MiB/core memory roofline.

The host merges the 8 (num, s, m) partials with a stable cross-shard
softmax combine in float64 and applies the final divide + sigmoid.
"""

from contextlib import ExitStack

import numpy as np

import concourse.bacc as bacc
import concourse.bass as bass
import concourse.mybir as mybir
import concourse.tile as tile
from concourse.bass_utils import run_bass_kernel_spmd

F32 = mybir.dt.float32

NCORES = 8
CAP = 262144
F = 512
SHARD = CAP // NCORES  # 32768

# Streaming geometry: one chunk = CHUNK_ROWS key/value rows; each SBUF
# partition holds CHUNK_ROWS/128 consecutive DRAM rows (contiguous
# per-partition DMA descriptors). 512 rows -> 1 MiB per DMA, and the PE's
# fp32 matmul bursts stay dense enough to hold the HAM clock at 2.4 GHz.
CHUNK_ROWS = 512

NACT = 4  # sub-tiles/chunk whose abs+reduce runs on ScalarE; rest on VectorE
KBUFS = 3
VBUFS = 3
DBUFS = 3
MARGIN = 20.0  # subtracted from the chunk-0 min (exp-overflow guard)


def _body(ctx, tc, q, k, v, ident, ones, out_vec, out_stats, shard_rows, chunk_rows):
    nc = tc.nc
    subt = chunk_rows // 128
    nchunk = shard_rows // chunk_rows
    ncols = shard_rows // 128
    nact = min(NACT, subt)

    singles = ctx.enter_context(tc.tile_pool(name="singles", bufs=1))
    kpool = ctx.enter_context(tc.tile_pool(name="kpool", bufs=KBUFS))
    vpool = ctx.enter_context(tc.tile_pool(name="vpool", bufs=VBUFS))
    dpool = ctx.enter_context(tc.tile_pool(name="dpool", bufs=DBUFS))
    apool = ctx.enter_context(tc.tile_pool(name="apool", bufs=4))
    pp = ctx.enter_context(tc.tile_pool(name="pp", bufs=1, space="PSUM"))

    # query broadcast to all 128 partitions (stride-0 partition DMA)
    qb = singles.tile([128, F], F32, tag="qb")
    nc.sync.dma_start(
        out=qb,
        in_=bass.AP(tensor=q.tensor, offset=q.offset, ap=[[0, 128]] + list(q.ap)),
    )
    id_sb = singles.tile([128, 128], F32, tag="id")
    nc.sync.dma_start(out=id_sb, in_=ident)
    ones_sb = singles.tile([1, 128], F32, tag="ones")
    nc.sync.dma_start(out=ones_sb, in_=ones)

    scores = singles.tile([128, ncols], F32, tag="scores")
    att = singles.tile([128, ncols], F32, tag="att")
    gmin = singles.tile([1, 1], F32, tag="gmin")
    gmin_col = singles.tile([128, 1], F32, tag="gmincol")

    kt = k.rearrange("(n p j) f -> n p j f", p=128, j=subt)
    vt = v.rearrange("(n p j) f -> n p j f", p=128, j=subt)

    acc = pp.tile([1, F], F32, tag="acc")
    for n in range(nchunk):
        kc = kpool.tile([128, subt, F], F32, tag="kc")
        nc.sync.dma_start(out=kc, in_=kt[n])
        vc = vpool.tile([128, subt, F], F32, tag="vc")
        nc.scalar.dma_start(out=vc, in_=vt[n])

        dchunk = dpool.tile([128, subt, F], F32, tag="dch")
        for j in range(subt):
            nc.vector.tensor_tensor(
                out=dchunk[:, j], in0=kc[:, j], in1=qb, op=mybir.AluOpType.subtract
            )
        col0 = n * subt
        for j in range(nact):
            a = apool.tile([128, F], F32, tag="a")
            nc.scalar.activation(
                out=a,
                in_=dchunk[:, j],
                func=mybir.ActivationFunctionType.Abs,
                accum_out=scores[:, col0 + j : col0 + j + 1],
            )
        if nact < subt:
            nc.vector.tensor_reduce(
                out=scores[:, col0 + nact : col0 + subt],
                in_=dchunk[:, nact:subt],
                axis=mybir.AxisListType.X,
                op=mybir.AluOpType.add,
                apply_absolute_value=True,
            )

        if n == 0:
            # softmax reference point: (cross-partition min of chunk 0) - MARGIN
            tmin = singles.tile([128, 1], F32, tag="tmin")
            nc.vector.tensor_reduce(
                out=tmin,
                in_=scores[:, 0:subt],
                axis=mybir.AxisListType.X,
                op=mybir.AluOpType.min,
            )
            trow_ps = pp.tile([1, 128], F32, tag="trow")
            nc.tensor.transpose(trow_ps, tmin, id_sb)
            nc.vector.tensor_reduce(
                out=gmin,
                in_=trow_ps,
                axis=mybir.AxisListType.X,
                op=mybir.AluOpType.min,
            )
            nc.vector.tensor_scalar_add(gmin, gmin, -MARGIN)
            # broadcast the scalar back to all partitions: ones.T @ gmin
            bc_ps = pp.tile([128, 1], F32, tag="bc")
            nc.tensor.matmul(bc_ps, lhsT=ones_sb, rhs=gmin, start=True, stop=True)
            nc.scalar.copy(out=gmin_col, in_=bc_ps)

        nc.scalar.activation(
            out=att[:, col0 : col0 + subt],
            in_=scores[:, col0 : col0 + subt],
            func=mybir.ActivationFunctionType.Exp,
            bias=gmin_col,
            scale=-1.0,
        )
        for j in range(subt):
            c = col0 + j
            nc.tensor.matmul(
                acc,
                lhsT=att[:, c : c + 1],
                rhs=vc[:, j],
                start=(c == 0),
                stop=(c == ncols - 1),
            )

    # ---- tail: s = sum(att), pack outputs ----
    scol = singles.tile([128, 1], F32, tag="scol")
    nc.vector.tensor_reduce(
        out=scol, in_=att, axis=mybir.AxisListType.X, op=mybir.AluOpType.add
    )
    srow_ps = pp.tile([1, 128], F32, tag="srow")
    nc.tensor.transpose(srow_ps, scol, id_sb)
    ssum = singles.tile([1, 1], F32, tag="ssum")
    nc.vector.tensor_reduce(
        out=ssum, in_=srow_ps, axis=mybir.AxisListType.X, op=mybir.AluOpType.add
    )
    out_sb = singles.tile([1, F], F32, tag="outsb")
    nc.vector.tensor_copy(out=out_sb, in_=acc)
    st_sb = singles.tile([1, 2], F32, tag="stsb")
    nc.vector.tensor_copy(out=st_sb[:, 0:1], in_=ssum)
    nc.vector.tensor_copy(out=st_sb[:, 1:2], in_=gmin)
    nc.sync.dma_start(out=out_vec, in_=out_sb)
    nc.sync.dma_start(out=out_stats, in_=st_sb)


def build_nc(shard_rows=SHARD, chunk_rows=CHUNK_ROWS, num_devices=NCORES, reps=1):
    nc = bacc.Bacc(
        "TRN2", target_bir_lowering=False, debug=False, num_devices=num_devices
    )
    q_h = nc.dram_tensor("query", [F], F32, kind="ExternalInput")
    k_h = nc.dram_tensor("keys", [shard_rows, F], F32, kind="ExternalInput")
    v_h = nc.dram_tensor("values", [shard_rows, F], F32, kind="ExternalInput")
    id_h = nc.dram_tensor("ident", [128, 128], F32, kind="ExternalInput")
    ones_h = nc.dram_tensor("ones_row", [1, 128], F32, kind="ExternalInput")
    onum_h = nc.dram_tensor("out_vec", [1, F], F32, kind="ExternalOutput")
    ostat_h = nc.dram_tensor("out_stats", [1, 2], F32, kind="ExternalOutput")

    with tile.TileContext(nc) as tc, ExitStack() as ctx:
        for _ in range(reps):
            with ExitStack() as rep_ctx:
                _body(
                    rep_ctx,
                    tc,
                    q_h.ap(),
                    k_h.ap(),
                    v_h.ap(),
                    id_h.ap(),
                    ones_h.ap(),
                    onum_h.ap(),
                    ostat_h.ap(),
                    shard_rows,
                    chunk_rows,
                )
    nc.compile()
    return nc


def make_in_maps(query, keys, values, shard_rows=SHARD, ncores=NCORES):
    query = np.ascontiguousarray(np.asarray(query), dtype=np.float32)
    keys = np.asarray(keys)
    values = np.asarray(values)
    ident = np.eye(128, dtype=np.float32)
    ones = np.ones((1, 128), dtype=np.float32)
    in_maps = []
    for i in range(ncores):
        sl = slice(i * shard_rows, (i + 1) * shard_rows)
        in_maps.append(
            {
                "query": query,
                "keys": np.ascontiguousarray(keys[sl], dtype=np.float32),
                "values": np.ascontiguousarray(values[sl], dtype=np.float32),
                "ident": ident,
                "ones_row": ones,
            }
        )
    return in_maps


def combine(results):
    """Merge per-core (num, s, m) partials: stable cross-shard softmax."""
    num = np.stack([np.asarray(r["out_vec"])[0] for r in results]).astype(np.float64)
    st = np.stack([np.asarray(r["out_stats"])[0] for r in results]).astype(np.float64)
    s, m = st[:, 0], st[:, 1]
    m0 = m.min()
    w = np.exp(m0 - m)  # <= 1
    vec = (num * w[:, None]).sum(axis=0) / (s * w).sum()
    return (1.0 / (1.0 + np.exp(-vec))).astype(np.float32)


_NC_CACHE = None


def kernel(query, keys, values):
    global _NC_CACHE
    if _NC_CACHE is None:
        _NC_CACHE = build_nc()
    in_maps = make_in_maps(query, keys, values)
    res = run_bass_kernel_spmd(_NC_CACHE, in_maps, core_ids=list(range(NCORES)))
    return combine(res.results)


if __name__ == "__main__":
    rng = np.random.default_rng(0)
    q = rng.standard_normal(F).astype(np.float32)
    k = rng.standard_normal((CAP, F)).astype(np.float32)
    v = rng.standard_normal((CAP, F)).astype(np.float32)
    out = kernel(q, k, v)
    print(out[:8])
